# revision 12
# baseline (speedup 1.0000x reference)
"""Trainium2 Bass kernel for a SiamRPN-style depthwise-xcorr head.

Computation (per batch sample):
  k = relu(bn(conv3x3(kernel, wk)))      # (256,7,7)  -> (256,5,5)
  s = relu(bn(conv3x3(search, ws)))      # (256,31,31)-> (256,29,29)
  f = depthwise_xcorr(s, k)              # (256,25,25)
  f = relu(bn(conv1x1(f, w1)))
  out = conv1x1(f, w2) + b2              # (256,25,25)

Sharding: data-parallel over batch, 8 samples per NeuronCore x 8 cores.
BN (eval mode) is folded into the conv weights on the host; the per-channel
shift is applied as the ScalarE activation bias during PSUM eviction.
The 3x3 convs are accumulated matmuls (9 taps x 2 cin tiles) over shifted
windows, 29-wide rows (no pad column).  The 25-tap depthwise xcorr is split
three ways, balanced so TensorE / VectorE / ScalarE all finish together:
  - N_PE taps as diag(k)-stationary matmuls on the tensor engine (diagonals
    built on VectorE with 2x-mode bf16 tensor_scalar against a bf16 identity),
  - N_DVE taps as a scalar_tensor_tensor accumulate chain on VectorE; the
    chain is *seeded* from the PE partial via STT ops whose in1 reads the
    xcorr PSUM directly (free merge of the PE partial),
  - the rest as ScalarE activation products (Copy with per-partition scale)
    folded into the VectorE chain with 2x-mode bf16 tensor_tensor adds.
Per-sample PE order is xcorr(s-1) -> search-conv(s) -> heads(s-1) so the
serial tap->heads dependency of sample s-1 hides under search-conv(s).
All matmul operands are bf16 (1 row/cycle); fp32 PSUM accumulation keeps
the conv reductions exact.
"""

import sys

if "/opt/trn_rl_repo" not in sys.path:
    sys.path.insert(0, "/opt/trn_rl_repo")

import ml_dtypes
import numpy as np

import concourse.bacc as bacc
import concourse.mybir as mybir
import concourse.tile as tile
from concourse.bass_utils import run_bass_kernel_spmd

EPS = 1e-5
B, CIN, H, SK, SS, COUT = 64, 256, 256, 7, 31, 256
NCORES = 8
NB = B // NCORES            # samples per core
OS = SS - 2                 # 29: search conv output width
OK = SK - 2                 # 5: kernel conv output width
OX = OS - OK + 1            # 25: xcorr output width
NPIX = OX * OX              # 625
SSP = SS + 1                # 32: padded search row width (DMA layout)
SKP = SK + 3                # 10: padded kernel row width
OSS = OS + 1                # 30: ss tile row stride (29 valid cols)
OKW = OK + 1                # 6:  kernel-conv matmul window width
NKF = NB * OK * OK          # 200: kf columns (5x5 taps per sample)

F32 = mybir.dt.float32
BF16 = mybir.dt.bfloat16

MUL = mybir.AluOpType.mult
ADD = mybir.AluOpType.add

# output-row chunks: each accumulated matmul's dst must sit inside one
# 512-f32 PSUM bank
S_CHUNKS = [(0, 17), (17, 12)]   # 17*29=493, 12*29=348
X_CHUNKS = [(0, 13), (13, 12)]   # 13*25=325, 12*25=300

# ---- xcorr tap split (per j-half; 25 taps total) -------------------------
# taps [0, n_pe) on the tensor engine, [n_pe, n_pe+n_dve) on VectorE
# (the first DVE tap doubles as the PSUM-seed merge), the rest as ScalarE
# products folded in by VectorE tensor_tensor adds.  The last sample has no
# next search-conv to hide behind, so its split leans on the tensor engine
# to shorten the drain tail.
SPLITS = [(8, 7)] * (NB - 1) + [(20, 5)]
N_PE_MAX = max(p for p, _ in SPLITS)

_CACHED = {}


def _build_nc():
    nc = bacc.Bacc("TRN2", target_bir_lowering=False, debug=False,
                   num_devices=NCORES)

    xs_d = nc.dram_tensor("xs", [NB, CIN, SS, SSP], BF16, kind="ExternalInput")
    xk_d = nc.dram_tensor("xk", [2, 128, NB * SK * SKP], BF16,
                          kind="ExternalInput")
    ws_d = nc.dram_tensor("ws", [128, 9, 2, 256], BF16, kind="ExternalInput")
    wk_d = nc.dram_tensor("wk", [128, 9, 2, 256], BF16, kind="ExternalInput")
    w1_d = nc.dram_tensor("w1", [128, 2, 256], BF16, kind="ExternalInput")
    w2_d = nc.dram_tensor("w2", [128, 2, 256], BF16, kind="ExternalInput")
    bb_d = nc.dram_tensor("bb", [128, 8], F32, kind="ExternalInput")
    idr_d = nc.dram_tensor("idr", [128, N_PE_MAX, 128], BF16,
                           kind="ExternalInput")
    y_d = nc.dram_tensor("y", [NB, COUT, NPIX], F32, kind="ExternalOutput")

    RELU = mybir.ActivationFunctionType.Relu
    IDENT = mybir.ActivationFunctionType.Identity
    COPY = mybir.ActivationFunctionType.Copy

    with tile.TileContext(nc) as tc:
        with (
            tc.tile_pool(name="wpool", bufs=1) as wpool,
            tc.tile_pool(name="xin", bufs=4) as xin,
            tc.tile_pool(name="smid", bufs=3) as smid,
            tc.tile_pool(name="dpool", bufs=6) as dpool,
            tc.tile_pool(name="fpool", bufs=2) as fpool,
            tc.tile_pool(name="ppool", bufs=2) as ppool,
            tc.tile_pool(name="opool", bufs=2) as opool,
            tc.tile_pool(name="ps_m", bufs=6, space="PSUM") as ps_m,
            tc.tile_pool(name="ps_h", bufs=2, space="PSUM") as ps_h,
        ):
            # ---- weight / constant loads.  kconv runs first on the PE, so
            # its inputs (bb, xk, wk) land first; ws follows tap-sliced.
            bb_t = wpool.tile([128, 8], F32, tag="bb")
            idr_t = wpool.tile([128, N_PE_MAX, 128], BF16, tag="idr")
            wk_t = wpool.tile([128, 9, 2, 256], BF16, tag="wk")
            xk_t = [wpool.tile([128, NB, SK * SKP], BF16, tag=f"xk{j}",
                               name=f"xk{j}") for j in range(2)]
            # kconv inputs (xk, wk, bb) first, striped over four queues so
            # the tensor engine can start ~1.5us in; ws follows tap-sliced,
            # idr/w1/w2 (not needed until diags/heads) last
            qs = [nc.sync, nc.gpsimd, nc.scalar]
            nc.scalar.dma_start(bb_t[:], bb_d[:])
            for j in range(2):
                qs[j].dma_start(xk_t[j][:], xk_d[j, :, :])
            for tc3 in range(3):
                qs[tc3].dma_start(
                    wk_t[:, 3 * tc3:3 * (tc3 + 1), :, :],
                    wk_d[:, 3 * tc3:3 * (tc3 + 1), :, :])
            ws_t = wpool.tile([128, 9, 2, 256], BF16, tag="ws")
            for t9 in range(9):
                qs[t9 % 3].dma_start(ws_t[:, t9:t9 + 1, :, :],
                                     ws_d[:, t9:t9 + 1, :, :])
            nc.scalar.dma_start(idr_t[:], idr_d[:])
            w1_t = wpool.tile([128, 2, 256], BF16, tag="w1")
            w2_t = wpool.tile([128, 2, 256], BF16, tag="w2")
            nc.gpsimd.dma_start(w1_t[:], w1_d[:])
            nc.gpsimd.dma_start(w2_t[:], w2_d[:])

            def bias(col):
                return bb_t[:, col:col + 1]

            # ---- kernel branch, all samples at once (N = 8*5*6 = 240) ----
            kf_t = [wpool.tile([128, NKF], F32, tag=f"kf{m}", name=f"kf{m}")
                    for m in range(2)]

            def kernel_conv():
                for m in range(2):
                    pk = ps_h.tile([128, 512], F32, tag="ph",
                                   name=f"pk{m}")
                    first = True
                    for t in range(9):
                        ky, kx = divmod(t, 3)
                        for j in range(2):
                            rhs = xk_t[j][:].rearrange(
                                "p s (a b) -> p s a b", a=SK, b=SKP)[
                                :, :, ky:ky + OK, kx:kx + OKW]
                            nc.tensor.matmul(
                                pk[:, 0:NB * OK * OKW],
                                wk_t[:, t, j, m * 128:(m + 1) * 128],
                                rhs, start=first, stop=(t == 8 and j == 1))
                            first = False
                    # strip the pad column while evicting: psum [s,5,6] ->
                    # kf [s,5,5]
                    nc.scalar.activation(
                        kf_t[m][:].rearrange("p (s a b) -> p s a b",
                                             s=NB, a=OK, b=OK),
                        pk[:, 0:NB * OK * OKW].rearrange(
                            "p (s a b) -> p s a b",
                            s=NB, a=OK, b=OKW)[:, :, :, 0:OK],
                        RELU, bias=bias(2 + m))

            def kcol(j, s, t):
                c = s * OK * OK + t
                return kf_t[j][:, c:c + 1]

            # ---- per-sample stages -----------------------------------------
            ss_all = {}
            dg_all = {}
            px_all = {}

            def search_conv(s, fillers=(), mid=None):
                # `fillers`: closures emitting one ScalarE product (+ its
                # VectorE fold) for sample s-1, interleaved after each
                # chunk-group eviction so the ScalarE queue (depth 0, HOL
                # blocking) never parks on a not-yet-ready eviction.
                fillers = list(fillers)
                xs_t = [xin.tile([128, SS, SSP], BF16, tag=f"xs{j}",
                                 name=f"xs{j}_{s}") for j in range(2)]
                for j in range(2):
                    nc.gpsimd.dma_start(
                        xs_t[j][:, 0:19, :],
                        xs_d[s, j * 128:(j + 1) * 128, 0:19, :])
                for j in range(2):
                    nc.gpsimd.dma_start(
                        xs_t[j][:, 19:SS, :],
                        xs_d[s, j * 128:(j + 1) * 128, 19:SS, :])
                ss_t = [smid.tile([128, OS, OSS], BF16, tag=f"ss{m}",
                                  name=f"ss{m}_{s}") for m in range(2)]
                grp = 0
                nfill = len(fillers)
                for ci, (r0, nr) in enumerate(S_CHUNKS):
                    for m in range(2):
                        psm = ps_m.tile([128, 512], F32, tag="ps",
                                        name=f"psm{m}_{ci}_{s}")
                        first = True
                        for t in range(9):
                            ky, kx = divmod(t, 3)
                            for j in range(2):
                                rhs = xs_t[j][:, r0 + ky:r0 + ky + nr,
                                              kx:kx + OS]
                                nc.tensor.matmul(
                                    psm[:, 0:nr * OS],
                                    ws_t[:, t, j, m * 128:(m + 1) * 128],
                                    rhs, start=first,
                                    stop=(t == 8 and j == 1))
                                first = False
                        # 29 valid cols into the 30-stride ss tile
                        nc.scalar.activation(
                            ss_t[m][:, r0:r0 + nr, 0:OS],
                            psm[:, 0:nr * OS].rearrange(
                                "p (a b) -> p a b", a=nr, b=OS),
                            RELU, bias=bias(m))
                        grp += 1
                        take = nfill * grp // 4 - nfill * (grp - 1) // 4
                        for _ in range(take):
                            fillers.pop(0)()
                        if grp == 2 and mid is not None:
                            ss_all[s] = ss_t
                            mid()
                for f in fillers:
                    f()
                ss_all[s] = ss_t

            def build_diags(s):
                # diagonals for sample s's PE taps: one VectorE tensor_tensor
                # against the replicated identity, k broadcast (stride-0 AP)
                # along the 128-wide inner dim
                n_pe = SPLITS[s][0]
                dg_t = [dpool.tile([128, N_PE_MAX, 128], BF16, tag=f"dg{j}",
                                   name=f"dg{j}_{s}") for j in range(2)]
                for j in range(2):
                    c0 = s * OK * OK
                    kb = kf_t[j][:, c0:c0 + n_pe].unsqueeze(2).broadcast_to(
                        [128, n_pe, 128])
                    nc.vector.tensor_tensor(
                        dg_t[j][:, 0:n_pe, :], idr_t[:, 0:n_pe, :], kb, MUL)
                dg_all[s] = dg_t

            def xcorr_pe(s, chunks=(0, 1)):
                # PE taps 0..n_pe-1 accumulate into per-chunk PSUMs per j
                n_pe = SPLITS[s][0]
                ss_t = ss_all[s]
                if s in dg_all:
                    dg_all[f"held{s}"] = dg_all.pop(s)
                dg_t = dg_all[f"held{s}"]
                px = px_all.setdefault(s, [[None, None], [None, None]])
                for ci in chunks:
                    r0, nr = X_CHUNKS[ci]
                    for j in range(2):
                        px[j][ci] = ps_m.tile([128, 512], F32, tag="ps",
                                              name=f"px{j}_{ci}_{s}")
                        for t in range(n_pe):
                            ky, kx = divmod(t, OK)
                            rhs = ss_t[j][:, r0 + ky:r0 + ky + nr,
                                          kx:kx + OX]
                            nc.tensor.matmul(
                                px[j][ci][:, 0:nr * OX],
                                dg_t[j][:, t, :],
                                rhs, start=(t == 0),
                                stop=(t == n_pe - 1))

            def xcorr_dve(s, acc_t, chunks=(0, 1), chunked_taps=False):
                # VectorE: seed the acc chain from the PE PSUM partial
                # (tap n_pe via STT with in1 = psum), then the remaining
                # DVE taps as in-place STT accumulates.  chunked_taps runs
                # the follow-up taps per chunk too (tail mode).
                n_pe, n_dve = SPLITS[s]
                ss_t = ss_all[s]
                px = px_all[s]
                for j in range(2):
                    ky, kx = divmod(n_pe, OK)
                    for ci in chunks:
                        r0, nr = X_CHUNKS[ci]
                        nc.vector.scalar_tensor_tensor(
                            acc_t[j][:, r0:r0 + nr, :],
                            ss_t[j][:, r0 + ky:r0 + ky + nr, kx:kx + OX],
                            kcol(j, s, n_pe),
                            px[j][ci][:, 0:nr * OX].rearrange(
                                "p (a b) -> p a b", a=nr, b=OX),
                            MUL, ADD)
                    for t in range(n_pe + 1, n_pe + n_dve):
                        ky, kx = divmod(t, OK)
                        if chunked_taps:
                            for ci in chunks:
                                r0, nr = X_CHUNKS[ci]
                                nc.vector.scalar_tensor_tensor(
                                    acc_t[j][:, r0:r0 + nr, :],
                                    ss_t[j][:, r0 + ky:r0 + ky + nr,
                                            kx:kx + OX],
                                    kcol(j, s, t),
                                    acc_t[j][:, r0:r0 + nr, :], MUL, ADD)
                        else:
                            nc.vector.scalar_tensor_tensor(
                                acc_t[j][:],
                                ss_t[j][:, ky:ky + OX, kx:kx + OX],
                                kcol(j, s, t), acc_t[j][:], MUL, ADD)
                return acc_t

            def xcorr_act(s, acc_t):
                # closures emitting one ScalarE product + its VectorE fold
                ss_t = ss_all.pop(s)
                fillers = []
                for j in range(2):
                    pr = [ppool.tile([128, OX, OX], BF16, tag=f"pr{j}{r}",
                                     name=f"pr{j}{r}_{s}") for r in range(2)]
                    n_pe, n_dve = SPLITS[s]
                    for i, t in enumerate(range(n_pe + n_dve, OK * OK)):
                        def emit(j=j, i=i, t=t, pr=pr):
                            ky, kx = divmod(t, OK)
                            nc.scalar.activation(
                                pr[i % 2][:],
                                ss_t[j][:, ky:ky + OX, kx:kx + OX],
                                COPY, scale=kcol(j, s, t))
                            nc.vector.tensor_tensor(
                                acc_t[j][:], acc_t[j][:], pr[i % 2][:], ADD)
                        fillers.append(emit)
                return fillers

            f2_all = {}

            def heads(s, acc_t, chunks=(0, 1)):
                # 1x1 conv -> bn+relu -> 1x1 conv + bias
                if s in f2_all:
                    f2_t = f2_all[s]
                else:
                    f2_t = [fpool.tile([128, OX, OX], BF16, tag=f"f2{m}",
                                       name=f"f2{m}_{s}") for m in range(2)]
                    f2_all[s] = f2_t
                for m in range(2):
                    for ci in chunks:
                        r0, nr = X_CHUNKS[ci]
                        ps1 = ps_h.tile([128, 512], F32, tag="ph",
                                        name=f"ps1{m}_{ci}_{s}")
                        for j in range(2):
                            nc.tensor.matmul(
                                ps1[:, 0:nr * OX],
                                w1_t[:, j, m * 128:(m + 1) * 128],
                                acc_t[j][:, r0:r0 + nr, :],
                                start=(j == 0), stop=(j == 1))
                        nc.scalar.activation(
                            f2_t[m][:, r0:r0 + nr, :],
                            ps1[:, 0:nr * OX],
                            RELU, bias=bias(4 + m))

                for m in range(2):
                    key = (s, m)
                    if key in f2_all:
                        out_t = f2_all[key]
                    else:
                        out_t = opool.tile([128, NPIX], F32, tag=f"o{m}",
                                           name=f"o{m}_{s}")
                        f2_all[key] = out_t
                    for ci in chunks:
                        r0, nr = X_CHUNKS[ci]
                        ps2 = ps_h.tile([128, 512], F32, tag="ph",
                                        name=f"ps2{m}_{ci}_{s}")
                        for j in range(2):
                            nc.tensor.matmul(
                                ps2[:, 0:nr * OX],
                                w2_t[:, j, m * 128:(m + 1) * 128],
                                f2_t[j][:, r0:r0 + nr, :],
                                start=(j == 0), stop=(j == 1))
                        nc.scalar.activation(
                            out_t[:, r0 * OX:(r0 + nr) * OX],
                            ps2[:, 0:nr * OX],
                            IDENT, bias=bias(6 + m))
                        nc.sync.dma_start(
                            y_d[s, m * 128:(m + 1) * 128,
                                r0 * OX:(r0 + nr) * OX],
                            out_t[:, r0 * OX:(r0 + nr) * OX])

            # ---- pipeline ---------------------------------------------------
            # PE period P_s: xcorr_pe(s-1) | search_conv(s) | heads(s-1)
            # VectorE:      diags(s) | seeds+taps(s-1) | product folds(s-1)
            # ScalarE:      ss evictions(s) | products(s-1) | f2/out evicts(s-1)
            def new_acc(s):
                return [fpool.tile([128, OX, OX], BF16, tag=f"ac{j}",
                                   name=f"ac{j}_{s}") for j in range(2)]

            kernel_conv()
            for d0 in range(5):
                build_diags(d0)
            search_conv(0)
            for s in range(1, NB - 1):
                xcorr_pe(s - 1)
                if s + 4 < NB:
                    build_diags(s + 4)
                acc_prev = xcorr_dve(s - 1, new_acc(s - 1))
                search_conv(s, xcorr_act(s - 1, acc_prev))
                heads(s - 1, acc_prev)

            # ---- tail: the last sample's xcorr is chunk-pipelined into its
            # own search conv (ss chunk c0 = rows 0..16 suffices for output
            # chunk c0), so the drain after the final matmul group is short.
            sl = NB - 1
            xcorr_pe(sl - 1)
            acc_prev = xcorr_dve(sl - 1, new_acc(sl - 1))
            acc_last = new_acc(sl)

            def tail_mid():
                xcorr_pe(sl, chunks=(0,))
                xcorr_dve(sl, acc_last, chunks=(0,), chunked_taps=True)
            search_conv(sl, xcorr_act(sl - 1, acc_prev), mid=tail_mid)
            heads(sl - 1, acc_prev)
            heads(sl, acc_last, chunks=(0,))
            xcorr_pe(sl, chunks=(1,))
            xcorr_dve(sl, acc_last, chunks=(1,), chunked_taps=True)
            for f in xcorr_act(sl, acc_last):
                f()
            heads(sl, acc_last, chunks=(1,))

    nc.compile()
    return nc


def _get_nc():
    if "nc" not in _CACHED:
        _CACHED["nc"] = _build_nc()
    return _CACHED["nc"]


def _fold_bn(w, g, b, m, v):
    scale = g / np.sqrt(v + EPS)
    return w * scale[:, None, None, None], (b - m * scale)


BFNP = ml_dtypes.bfloat16


def _pack3x3(w):
    t = w.transpose(2, 3, 1, 0).reshape(9, 2, 128, 256)  # t, j, p, c
    return np.ascontiguousarray(t.transpose(2, 0, 1, 3).astype(BFNP))


def _pack1x1(w):
    t = w[:, :, 0, 0].T.reshape(2, 128, 256)             # j, p, c
    return np.ascontiguousarray(t.transpose(1, 0, 2).astype(BFNP))


def _make_in_maps(kernel, search, wk, gk, bk, mk, vk, ws, gs, bs, ms, vs,
                  w1, g1, b1, m1, v1, w2, b2):
    wk_f, bk_f = _fold_bn(wk, gk, bk, mk, vk)
    ws_f, bs_f = _fold_bn(ws, gs, bs, ms, vs)
    w1_f, b1_f = _fold_bn(w1, g1, b1, m1, v1)

    xs = np.zeros((B, CIN, SS, SSP), BFNP)
    xs[:, :, :, :SS] = search.astype(BFNP)
    xkp = np.zeros((B, CIN, SK, SKP), BFNP)
    xkp[:, :, :, :SK] = kernel.astype(BFNP)
    # [2, 128, NB*70]: partition line holds all samples of one core
    xkp = xkp.reshape(B, CIN, SK * SKP)

    # bias columns: [bs0, bs1, bk0, bk1, b10, b11, b20, b21]
    bb = np.stack([bs_f[:128], bs_f[128:], bk_f[:128], bk_f[128:],
                   b1_f[:128], b1_f[128:],
                   np.asarray(b2)[:128], np.asarray(b2)[128:]],
                  axis=1).astype(np.float32)

    common = dict(
        ws=_pack3x3(ws_f), wk=_pack3x3(wk_f),
        w1=_pack1x1(w1_f), w2=_pack1x1(w2),
        bb=np.ascontiguousarray(bb),
        idr=np.ascontiguousarray(np.broadcast_to(
            np.eye(128, dtype=BFNP), (N_PE_MAX, 128, 128)
        ).transpose(1, 0, 2)),
    )
    in_maps = []
    for c in range(NCORES):
        sl = slice(c * NB, (c + 1) * NB)
        xk_core = xkp[sl].reshape(NB, 2, 128, SK * SKP)
        xk_core = np.ascontiguousarray(
            xk_core.transpose(1, 2, 0, 3).reshape(2, 128, NB * SK * SKP))
        in_maps.append(dict(xs=np.ascontiguousarray(xs[sl]),
                            xk=xk_core, **common))
    return in_maps


def kernel(**inputs):
    in_maps = _make_in_maps(**inputs)
    nc = _get_nc()
    res = run_bass_kernel_spmd(nc, in_maps, core_ids=list(range(NCORES)))
    out = np.concatenate([r["y"] for r in res.results], axis=0)
    return out.reshape(B, COUT, OX, OX).astype(np.float32)


# revision 13
# speedup vs baseline: 1.0022x; 1.0022x over previous
"""Trainium2 Bass kernel for a SiamRPN-style depthwise-xcorr head.

Computation (per batch sample):
  k = relu(bn(conv3x3(kernel, wk)))      # (256,7,7)  -> (256,5,5)
  s = relu(bn(conv3x3(search, ws)))      # (256,31,31)-> (256,29,29)
  f = depthwise_xcorr(s, k)              # (256,25,25)
  f = relu(bn(conv1x1(f, w1)))
  out = conv1x1(f, w2) + b2              # (256,25,25)

Sharding: data-parallel over batch, 8 samples per NeuronCore x 8 cores.
BN (eval mode) is folded into the conv weights on the host; the per-channel
shift is applied as the ScalarE activation bias during PSUM eviction.
The 3x3 convs are accumulated matmuls (9 taps x 2 cin tiles) over shifted
windows, 29-wide rows (no pad column).  The 25-tap depthwise xcorr is split
three ways, balanced so TensorE / VectorE / ScalarE all finish together:
  - N_PE taps as diag(k)-stationary matmuls on the tensor engine (diagonals
    built on VectorE with 2x-mode bf16 tensor_scalar against a bf16 identity),
  - N_DVE taps as a scalar_tensor_tensor accumulate chain on VectorE; the
    chain is *seeded* from the PE partial via STT ops whose in1 reads the
    xcorr PSUM directly (free merge of the PE partial),
  - the rest as ScalarE activation products (Copy with per-partition scale)
    folded into the VectorE chain with 2x-mode bf16 tensor_tensor adds.
Per-sample PE order is xcorr(s-1) -> search-conv(s) -> heads(s-1) so the
serial tap->heads dependency of sample s-1 hides under search-conv(s).
All matmul operands are bf16 (1 row/cycle); fp32 PSUM accumulation keeps
the conv reductions exact.
"""

import sys

if "/opt/trn_rl_repo" not in sys.path:
    sys.path.insert(0, "/opt/trn_rl_repo")

import ml_dtypes
import numpy as np

import concourse.bacc as bacc
import concourse.mybir as mybir
import concourse.tile as tile
from concourse.bass_utils import run_bass_kernel_spmd

EPS = 1e-5
B, CIN, H, SK, SS, COUT = 64, 256, 256, 7, 31, 256
NCORES = 8
NB = B // NCORES            # samples per core
OS = SS - 2                 # 29: search conv output width
OK = SK - 2                 # 5: kernel conv output width
OX = OS - OK + 1            # 25: xcorr output width
NPIX = OX * OX              # 625
SSP = SS + 1                # 32: padded search row width (DMA layout)
SKP = SK + 3                # 10: padded kernel row width
OSS = OS + 1                # 30: ss tile row stride (29 valid cols)
OKW = OK + 1                # 6:  kernel-conv matmul window width
NKF = NB * OK * OK          # 200: kf columns (5x5 taps per sample)

F32 = mybir.dt.float32
BF16 = mybir.dt.bfloat16

MUL = mybir.AluOpType.mult
ADD = mybir.AluOpType.add

# output-row chunks: each accumulated matmul's dst must sit inside one
# 512-f32 PSUM bank
S_CHUNKS = [(0, 17), (17, 12)]   # 17*29=493, 12*29=348
X_CHUNKS = [(0, 13), (13, 12)]   # 13*25=325, 12*25=300

# ---- xcorr tap split (per j-half; 25 taps total) -------------------------
# taps [0, n_pe) on the tensor engine, [n_pe, n_pe+n_dve) on VectorE
# (the first DVE tap doubles as the PSUM-seed merge), the rest as ScalarE
# products folded in by VectorE tensor_tensor adds.  The last sample has no
# next search-conv to hide behind, so its split leans on the tensor engine
# to shorten the drain tail.
SPLITS = [(8, 7)] * (NB - 1) + [(20, 5)]
N_PE_MAX = max(p for p, _ in SPLITS)

_CACHED = {}


def _build_nc():
    nc = bacc.Bacc("TRN2", target_bir_lowering=False, debug=False,
                   num_devices=NCORES)

    xs_d = nc.dram_tensor("xs", [NB, CIN, SS, SSP], BF16, kind="ExternalInput")
    xk_d = nc.dram_tensor("xk", [2, 128, NB * SK * SKP], BF16,
                          kind="ExternalInput")
    ws_d = nc.dram_tensor("ws", [128, 9, 2, 256], BF16, kind="ExternalInput")
    wk_d = nc.dram_tensor("wk", [128, 9, 2, 256], BF16, kind="ExternalInput")
    w1_d = nc.dram_tensor("w1", [128, 2, 256], BF16, kind="ExternalInput")
    w2_d = nc.dram_tensor("w2", [128, 2, 256], BF16, kind="ExternalInput")
    bb_d = nc.dram_tensor("bb", [128, 8], F32, kind="ExternalInput")
    idr_d = nc.dram_tensor("idr", [128, N_PE_MAX, 128], BF16,
                           kind="ExternalInput")
    y_d = nc.dram_tensor("y", [NB, COUT, NPIX], F32, kind="ExternalOutput")

    RELU = mybir.ActivationFunctionType.Relu
    IDENT = mybir.ActivationFunctionType.Identity
    COPY = mybir.ActivationFunctionType.Copy

    with tile.TileContext(nc) as tc:
        with (
            tc.tile_pool(name="wpool", bufs=1) as wpool,
            tc.tile_pool(name="xin", bufs=4) as xin,
            tc.tile_pool(name="smid", bufs=3) as smid,
            tc.tile_pool(name="dpool", bufs=4) as dpool,
            tc.tile_pool(name="fpool", bufs=2) as fpool,
            tc.tile_pool(name="ppool", bufs=2) as ppool,
            tc.tile_pool(name="opool", bufs=2) as opool,
            tc.tile_pool(name="ps_m", bufs=6, space="PSUM") as ps_m,
            tc.tile_pool(name="ps_h", bufs=2, space="PSUM") as ps_h,
        ):
            # ---- weight / constant loads.  kconv runs first on the PE, so
            # its inputs (bb, xk, wk) land first; ws follows tap-sliced.
            bb_t = wpool.tile([128, 8], F32, tag="bb")
            idr_t = wpool.tile([128, N_PE_MAX, 128], BF16, tag="idr")
            wk_t = wpool.tile([128, 9, 2, 256], BF16, tag="wk")
            xk_t = [wpool.tile([128, NB, SK * SKP], BF16, tag=f"xk{j}",
                               name=f"xk{j}") for j in range(2)]
            # kconv inputs (xk, wk, bb) first, striped over four queues so
            # the tensor engine can start ~1.5us in; ws follows tap-sliced,
            # idr/w1/w2 (not needed until diags/heads) last
            qs = [nc.sync, nc.gpsimd, nc.scalar]
            nc.scalar.dma_start(bb_t[:], bb_d[:])
            for j in range(2):
                qs[j].dma_start(xk_t[j][:], xk_d[j, :, :])
            for tc3 in range(3):
                qs[tc3].dma_start(
                    wk_t[:, 3 * tc3:3 * (tc3 + 1), :, :],
                    wk_d[:, 3 * tc3:3 * (tc3 + 1), :, :])
            ws_t = wpool.tile([128, 9, 2, 256], BF16, tag="ws")
            for t9 in range(9):
                qs[t9 % 3].dma_start(ws_t[:, t9:t9 + 1, :, :],
                                     ws_d[:, t9:t9 + 1, :, :])
            nc.scalar.dma_start(idr_t[:], idr_d[:])
            w1_t = wpool.tile([128, 2, 256], BF16, tag="w1")
            w2_t = wpool.tile([128, 2, 256], BF16, tag="w2")
            nc.gpsimd.dma_start(w1_t[:], w1_d[:])
            nc.gpsimd.dma_start(w2_t[:], w2_d[:])

            def bias(col):
                return bb_t[:, col:col + 1]

            # ---- kernel branch, all samples at once (N = 8*5*6 = 240) ----
            kf_t = [wpool.tile([128, NKF], F32, tag=f"kf{m}", name=f"kf{m}")
                    for m in range(2)]

            def kernel_conv():
                for m in range(2):
                    pk = ps_h.tile([128, 512], F32, tag="ph",
                                   name=f"pk{m}")
                    first = True
                    for t in range(9):
                        ky, kx = divmod(t, 3)
                        for j in range(2):
                            rhs = xk_t[j][:].rearrange(
                                "p s (a b) -> p s a b", a=SK, b=SKP)[
                                :, :, ky:ky + OK, kx:kx + OKW]
                            nc.tensor.matmul(
                                pk[:, 0:NB * OK * OKW],
                                wk_t[:, t, j, m * 128:(m + 1) * 128],
                                rhs, start=first, stop=(t == 8 and j == 1))
                            first = False
                    # strip the pad column while evicting: psum [s,5,6] ->
                    # kf [s,5,5]
                    nc.scalar.activation(
                        kf_t[m][:].rearrange("p (s a b) -> p s a b",
                                             s=NB, a=OK, b=OK),
                        pk[:, 0:NB * OK * OKW].rearrange(
                            "p (s a b) -> p s a b",
                            s=NB, a=OK, b=OKW)[:, :, :, 0:OK],
                        RELU, bias=bias(2 + m))

            def kcol(j, s, t):
                c = s * OK * OK + t
                return kf_t[j][:, c:c + 1]

            # ---- per-sample stages -----------------------------------------
            ss_all = {}
            dg_all = {}
            px_all = {}

            def search_conv(s, fillers=(), mid=None):
                # `fillers`: closures emitting one ScalarE product (+ its
                # VectorE fold) for sample s-1, interleaved after each
                # chunk-group eviction so the ScalarE queue (depth 0, HOL
                # blocking) never parks on a not-yet-ready eviction.
                fillers = list(fillers)
                xs_t = [xin.tile([128, SS, SSP], BF16, tag=f"xs{j}",
                                 name=f"xs{j}_{s}") for j in range(2)]
                for j in range(2):
                    nc.gpsimd.dma_start(
                        xs_t[j][:, 0:19, :],
                        xs_d[s, j * 128:(j + 1) * 128, 0:19, :])
                for j in range(2):
                    nc.gpsimd.dma_start(
                        xs_t[j][:, 19:SS, :],
                        xs_d[s, j * 128:(j + 1) * 128, 19:SS, :])
                ss_t = [smid.tile([128, OS, OSS], BF16, tag=f"ss{m}",
                                  name=f"ss{m}_{s}") for m in range(2)]
                grp = 0
                nfill = len(fillers)
                for ci, (r0, nr) in enumerate(S_CHUNKS):
                    for m in range(2):
                        psm = ps_m.tile([128, 512], F32, tag="ps",
                                        name=f"psm{m}_{ci}_{s}")
                        first = True
                        for t in range(9):
                            ky, kx = divmod(t, 3)
                            for j in range(2):
                                rhs = xs_t[j][:, r0 + ky:r0 + ky + nr,
                                              kx:kx + OS]
                                nc.tensor.matmul(
                                    psm[:, 0:nr * OS],
                                    ws_t[:, t, j, m * 128:(m + 1) * 128],
                                    rhs, start=first,
                                    stop=(t == 8 and j == 1))
                                first = False
                        # 29 valid cols into the 30-stride ss tile
                        nc.scalar.activation(
                            ss_t[m][:, r0:r0 + nr, 0:OS],
                            psm[:, 0:nr * OS].rearrange(
                                "p (a b) -> p a b", a=nr, b=OS),
                            RELU, bias=bias(m))
                        grp += 1
                        take = nfill * grp // 4 - nfill * (grp - 1) // 4
                        for _ in range(take):
                            fillers.pop(0)()
                        if grp == 2 and mid is not None:
                            ss_all[s] = ss_t
                            mid()
                for f in fillers:
                    f()
                ss_all[s] = ss_t

            def build_diags(s):
                # diagonals for sample s's PE taps: one VectorE tensor_tensor
                # against the replicated identity, k broadcast (stride-0 AP)
                # along the 128-wide inner dim
                n_pe = SPLITS[s][0]
                dg_t = [dpool.tile([128, N_PE_MAX, 128], BF16, tag=f"dg{j}",
                                   name=f"dg{j}_{s}") for j in range(2)]
                for j in range(2):
                    c0 = s * OK * OK
                    kb = kf_t[j][:, c0:c0 + n_pe].unsqueeze(2).broadcast_to(
                        [128, n_pe, 128])
                    nc.vector.tensor_tensor(
                        dg_t[j][:, 0:n_pe, :], idr_t[:, 0:n_pe, :], kb, MUL)
                dg_all[s] = dg_t

            def xcorr_pe(s, chunks=(0, 1)):
                # PE taps 0..n_pe-1 accumulate into per-chunk PSUMs per j
                n_pe = SPLITS[s][0]
                ss_t = ss_all[s]
                if s in dg_all:
                    dg_all[f"held{s}"] = dg_all.pop(s)
                dg_t = dg_all[f"held{s}"]
                px = px_all.setdefault(s, [[None, None], [None, None]])
                for ci in chunks:
                    r0, nr = X_CHUNKS[ci]
                    for j in range(2):
                        px[j][ci] = ps_m.tile([128, 512], F32, tag="ps",
                                              name=f"px{j}_{ci}_{s}")
                        for t in range(n_pe):
                            ky, kx = divmod(t, OK)
                            rhs = ss_t[j][:, r0 + ky:r0 + ky + nr,
                                          kx:kx + OX]
                            nc.tensor.matmul(
                                px[j][ci][:, 0:nr * OX],
                                dg_t[j][:, t, :],
                                rhs, start=(t == 0),
                                stop=(t == n_pe - 1))

            def xcorr_dve(s, acc_t, chunks=(0, 1), chunked_taps=False):
                # VectorE: seed the acc chain from the PE PSUM partial
                # (tap n_pe via STT with in1 = psum), then the remaining
                # DVE taps as in-place STT accumulates.  chunked_taps runs
                # the follow-up taps per chunk too (tail mode).
                n_pe, n_dve = SPLITS[s]
                ss_t = ss_all[s]
                px = px_all[s]
                for j in range(2):
                    ky, kx = divmod(n_pe, OK)
                    for ci in chunks:
                        r0, nr = X_CHUNKS[ci]
                        nc.vector.scalar_tensor_tensor(
                            acc_t[j][:, r0:r0 + nr, :],
                            ss_t[j][:, r0 + ky:r0 + ky + nr, kx:kx + OX],
                            kcol(j, s, n_pe),
                            px[j][ci][:, 0:nr * OX].rearrange(
                                "p (a b) -> p a b", a=nr, b=OX),
                            MUL, ADD)
                    for t in range(n_pe + 1, n_pe + n_dve):
                        ky, kx = divmod(t, OK)
                        if chunked_taps:
                            for ci in chunks:
                                r0, nr = X_CHUNKS[ci]
                                nc.vector.scalar_tensor_tensor(
                                    acc_t[j][:, r0:r0 + nr, :],
                                    ss_t[j][:, r0 + ky:r0 + ky + nr,
                                            kx:kx + OX],
                                    kcol(j, s, t),
                                    acc_t[j][:, r0:r0 + nr, :], MUL, ADD)
                        else:
                            nc.vector.scalar_tensor_tensor(
                                acc_t[j][:],
                                ss_t[j][:, ky:ky + OX, kx:kx + OX],
                                kcol(j, s, t), acc_t[j][:], MUL, ADD)
                return acc_t

            def xcorr_act(s, acc_t):
                # closures emitting one ScalarE product + its VectorE fold
                ss_t = ss_all.pop(s)
                fillers = []
                for j in range(2):
                    pr = [ppool.tile([128, OX, OX], BF16, tag=f"pr{j}{r}",
                                     name=f"pr{j}{r}_{s}") for r in range(2)]
                    n_pe, n_dve = SPLITS[s]
                    for i, t in enumerate(range(n_pe + n_dve, OK * OK)):
                        def emit(j=j, i=i, t=t, pr=pr):
                            ky, kx = divmod(t, OK)
                            nc.scalar.activation(
                                pr[i % 2][:],
                                ss_t[j][:, ky:ky + OX, kx:kx + OX],
                                COPY, scale=kcol(j, s, t))
                            nc.vector.tensor_tensor(
                                acc_t[j][:], acc_t[j][:], pr[i % 2][:], ADD)
                        fillers.append(emit)
                return fillers

            f2_all = {}

            def heads(s, acc_t, chunks=(0, 1)):
                # 1x1 conv -> bn+relu -> 1x1 conv + bias
                if s in f2_all:
                    f2_t = f2_all[s]
                else:
                    f2_t = [fpool.tile([128, OX, OX], BF16, tag=f"f2{m}",
                                       name=f"f2{m}_{s}") for m in range(2)]
                    f2_all[s] = f2_t
                for m in range(2):
                    for ci in chunks:
                        r0, nr = X_CHUNKS[ci]
                        ps1 = ps_h.tile([128, 512], F32, tag="ph",
                                        name=f"ps1{m}_{ci}_{s}")
                        for j in range(2):
                            nc.tensor.matmul(
                                ps1[:, 0:nr * OX],
                                w1_t[:, j, m * 128:(m + 1) * 128],
                                acc_t[j][:, r0:r0 + nr, :],
                                start=(j == 0), stop=(j == 1))
                        nc.scalar.activation(
                            f2_t[m][:, r0:r0 + nr, :],
                            ps1[:, 0:nr * OX],
                            RELU, bias=bias(4 + m))

                for m in range(2):
                    key = (s, m)
                    if key in f2_all:
                        out_t = f2_all[key]
                    else:
                        out_t = opool.tile([128, NPIX], F32, tag=f"o{m}",
                                           name=f"o{m}_{s}")
                        f2_all[key] = out_t
                    for ci in chunks:
                        r0, nr = X_CHUNKS[ci]
                        ps2 = ps_h.tile([128, 512], F32, tag="ph",
                                        name=f"ps2{m}_{ci}_{s}")
                        for j in range(2):
                            nc.tensor.matmul(
                                ps2[:, 0:nr * OX],
                                w2_t[:, j, m * 128:(m + 1) * 128],
                                f2_t[j][:, r0:r0 + nr, :],
                                start=(j == 0), stop=(j == 1))
                        nc.scalar.activation(
                            out_t[:, r0 * OX:(r0 + nr) * OX],
                            ps2[:, 0:nr * OX],
                            IDENT, bias=bias(6 + m))
                        nc.sync.dma_start(
                            y_d[s, m * 128:(m + 1) * 128,
                                r0 * OX:(r0 + nr) * OX],
                            out_t[:, r0 * OX:(r0 + nr) * OX])

            # ---- pipeline ---------------------------------------------------
            # PE period P_s: xcorr_pe(s-1) | search_conv(s) | heads(s-1)
            # VectorE:      diags(s) | seeds+taps(s-1) | product folds(s-1)
            # ScalarE:      ss evictions(s) | products(s-1) | f2/out evicts(s-1)
            def new_acc(s):
                return [fpool.tile([128, OX, OX], BF16, tag=f"ac{j}",
                                   name=f"ac{j}_{s}") for j in range(2)]

            kernel_conv()
            build_diags(0)
            build_diags(1)
            build_diags(2)
            search_conv(0)
            for s in range(1, NB - 1):
                xcorr_pe(s - 1)
                if s + 2 < NB:
                    build_diags(s + 2)
                acc_prev = xcorr_dve(s - 1, new_acc(s - 1))
                search_conv(s, xcorr_act(s - 1, acc_prev))
                heads(s - 1, acc_prev)

            # ---- tail: the last sample's xcorr is chunk-pipelined into its
            # own search conv (ss chunk c0 = rows 0..16 suffices for output
            # chunk c0), so the drain after the final matmul group is short.
            sl = NB - 1
            xcorr_pe(sl - 1)
            acc_prev = xcorr_dve(sl - 1, new_acc(sl - 1))
            acc_last = new_acc(sl)

            def tail_mid():
                xcorr_pe(sl, chunks=(0,))
                xcorr_dve(sl, acc_last, chunks=(0,), chunked_taps=True)
            search_conv(sl, xcorr_act(sl - 1, acc_prev), mid=tail_mid)
            heads(sl - 1, acc_prev)
            xcorr_pe(sl, chunks=(1,))
            xcorr_dve(sl, acc_last, chunks=(1,), chunked_taps=True)
            for f in xcorr_act(sl, acc_last):
                f()
            heads(sl, acc_last)

    nc.compile()
    return nc


def _get_nc():
    if "nc" not in _CACHED:
        _CACHED["nc"] = _build_nc()
    return _CACHED["nc"]


def _fold_bn(w, g, b, m, v):
    scale = g / np.sqrt(v + EPS)
    return w * scale[:, None, None, None], (b - m * scale)


BFNP = ml_dtypes.bfloat16


def _pack3x3(w):
    t = w.transpose(2, 3, 1, 0).reshape(9, 2, 128, 256)  # t, j, p, c
    return np.ascontiguousarray(t.transpose(2, 0, 1, 3).astype(BFNP))


def _pack1x1(w):
    t = w[:, :, 0, 0].T.reshape(2, 128, 256)             # j, p, c
    return np.ascontiguousarray(t.transpose(1, 0, 2).astype(BFNP))


def _make_in_maps(kernel, search, wk, gk, bk, mk, vk, ws, gs, bs, ms, vs,
                  w1, g1, b1, m1, v1, w2, b2):
    wk_f, bk_f = _fold_bn(wk, gk, bk, mk, vk)
    ws_f, bs_f = _fold_bn(ws, gs, bs, ms, vs)
    w1_f, b1_f = _fold_bn(w1, g1, b1, m1, v1)

    xs = np.zeros((B, CIN, SS, SSP), BFNP)
    xs[:, :, :, :SS] = search.astype(BFNP)
    xkp = np.zeros((B, CIN, SK, SKP), BFNP)
    xkp[:, :, :, :SK] = kernel.astype(BFNP)
    # [2, 128, NB*70]: partition line holds all samples of one core
    xkp = xkp.reshape(B, CIN, SK * SKP)

    # bias columns: [bs0, bs1, bk0, bk1, b10, b11, b20, b21]
    bb = np.stack([bs_f[:128], bs_f[128:], bk_f[:128], bk_f[128:],
                   b1_f[:128], b1_f[128:],
                   np.asarray(b2)[:128], np.asarray(b2)[128:]],
                  axis=1).astype(np.float32)

    common = dict(
        ws=_pack3x3(ws_f), wk=_pack3x3(wk_f),
        w1=_pack1x1(w1_f), w2=_pack1x1(w2),
        bb=np.ascontiguousarray(bb),
        idr=np.ascontiguousarray(np.broadcast_to(
            np.eye(128, dtype=BFNP), (N_PE_MAX, 128, 128)
        ).transpose(1, 0, 2)),
    )
    in_maps = []
    for c in range(NCORES):
        sl = slice(c * NB, (c + 1) * NB)
        xk_core = xkp[sl].reshape(NB, 2, 128, SK * SKP)
        xk_core = np.ascontiguousarray(
            xk_core.transpose(1, 2, 0, 3).reshape(2, 128, NB * SK * SKP))
        in_maps.append(dict(xs=np.ascontiguousarray(xs[sl]),
                            xk=xk_core, **common))
    return in_maps


def kernel(**inputs):
    in_maps = _make_in_maps(**inputs)
    nc = _get_nc()
    res = run_bass_kernel_spmd(nc, in_maps, core_ids=list(range(NCORES)))
    out = np.concatenate([r["y"] for r in res.results], axis=0)
    return out.reshape(B, COUT, OX, OX).astype(np.float32)
